# revision 14
# baseline (speedup 1.0000x reference)
"""ChannelGroupConv (1x1 conv, block-lower-triangular channel mask) on 8 TRN2 cores.

out[b, co, h, w] = sum_ci maskedW[co, ci] * x[b, ci, h, w] + bias[co]

Sharding: data-parallel over H — core i handles rows [i*64, (i+1)*64) of every
batch. The masked weight (compile-time constant mask, applied on host) and the
bias are replicated.

Per-core kernel: stream [128, TILE] pixel tiles through one stationary
128x128 matmul per 512-px slice, bias-add PSUM->SBUF, stream out.

The matmul runs in float32r (same 4-byte fp32 data, single-pass PE multiply):
measured on HW, plain fp32 matmuls stream at 4 cycles/row and never release
the PE HAM clock throttle (1.2 GHz), capping PE at ~900 us/core -- 2.8x the
~320 us DMA roofline. float32r streams at 1 cycle/row (~150 us/core), putting
the kernel back on the memory roofline. Measured output rel err vs f64: 1.6e-4.
"""

import numpy as np

import concourse.bass as bass
import concourse.mybir as mybir
from concourse import bacc
from concourse.tile import TileContext
from concourse.bass_utils import run_bass_kernel_spmd

N_CORES = 8
B, C, H, W = 4, 128, 512, 512
NGROUP, CIN, COUT = 16, 8, 8
H_SH = H // N_CORES          # 64 rows per core
PIX = H_SH * W               # 32768 pixels per batch per core
TILE = 8192                  # f32 cols per DMA tile (32KB/partition, 4MB/DMA)
MM_N = 512                   # matmul free dim (one PSUM bank, fp32 max)
HALF = C // 2

_CACHE = {}


def _build_nc(repeat=1):
    key = ("nc", repeat)
    if key in _CACHE:
        return _CACHE[key]
    nc = bacc.Bacc()
    f32 = mybir.dt.float32
    f32r = mybir.dt.float32r
    x_d = nc.declare_dram_parameter("x", [B, C, PIX], f32r, isOutput=False)
    w_d = nc.declare_dram_parameter("wT", [C, C], f32r, isOutput=False)
    b_d = nc.declare_dram_parameter("bias", [C, 1], f32, isOutput=False)
    o_d = nc.declare_dram_parameter("out", [B, C, PIX], f32, isOutput=True)

    with TileContext(nc) as tc:
        with (
            tc.tile_pool(name="const", bufs=1) as cpool,
            tc.tile_pool(name="xin", bufs=2) as xpool,
            tc.tile_pool(name="oout", bufs=2) as opool,
            tc.tile_pool(name="ps", bufs=8, space="PSUM") as ppool,
        ):
            wt = cpool.tile([C, C], f32r)
            nc.sync.dma_start(out=wt, in_=w_d[:, :])
            bt = cpool.tile([C, 1], f32)
            nc.sync.dma_start(out=bt, in_=b_d[:, :])
            for _rep in range(repeat):
                for b in range(B):
                    for t in range(PIX // TILE):
                        xt = xpool.tile([C, TILE], f32r)
                        nc.sync.dma_start(
                            out=xt, in_=x_d[b, :, t * TILE:(t + 1) * TILE]
                        )
                        ot = opool.tile([C, TILE], f32)
                        for s in range(TILE // MM_N):
                            sl = slice(s * MM_N, (s + 1) * MM_N)
                            ps = ppool.tile([C, MM_N], f32)
                            nc.tensor.matmul(
                                ps, wt, xt[:, sl], start=True, stop=True
                            )
                            nc.any.tensor_scalar_add(ot[:, sl], ps, bt)
                        nc.sync.dma_start(
                            out=o_d[b, :, t * TILE:(t + 1) * TILE], in_=ot
                        )
    nc.finalize()
    _CACHE["nc"] = nc
    return nc


def _masked_wT(weight):
    go = np.arange(NGROUP * COUT) // COUT
    gi = np.arange(NGROUP * CIN) // CIN
    mask = (gi[None, :] <= go[:, None]).astype(np.float32)
    wt = weight.reshape(C, C) * mask          # [Cout, Cin]
    return np.ascontiguousarray(wt.T)         # [Cin, Cout] = lhsT


def kernel(x, weight, bias, _trace=False):
    x = np.asarray(x, dtype=np.float32)
    weight = np.asarray(weight, dtype=np.float32)
    bias = np.asarray(bias, dtype=np.float32)

    nc = _build_nc()
    wT = _masked_wT(weight)
    b2 = np.ascontiguousarray(bias.reshape(C, 1))

    in_maps = []
    for i in range(N_CORES):
        shard = np.ascontiguousarray(x[:, :, i * H_SH:(i + 1) * H_SH, :])
        in_maps.append({"x": shard.reshape(B, C, PIX), "wT": wT, "bias": b2})

    res = run_bass_kernel_spmd(nc, in_maps, core_ids=list(range(N_CORES)))

    out = np.empty((B, C, H, W), dtype=np.float32)
    for i in range(N_CORES):
        out[:, :, i * H_SH:(i + 1) * H_SH, :] = res.results[i]["out"].reshape(
            B, C, H_SH, W
        )
    return out


# revision 18
# speedup vs baseline: 1.4160x; 1.4160x over previous
"""ChannelGroupConv (1x1 conv, block-lower-triangular channel mask) on 8 TRN2 cores.

out[b, co, h, w] = sum_ci maskedW[co, ci] * x[b, ci, h, w] + bias[co]

Sharding: data-parallel over H — core i handles rows [i*64, (i+1)*64) of every
batch. The masked weight (compile-time constant mask, applied on host) and the
bias are replicated.

Per-core kernel: stream [128, TILE] pixel tiles through one stationary
128x128 matmul per 512-px slice, bias-add PSUM->SBUF, stream out.

The matmul runs in float32r (same 4-byte fp32 data, single-pass PE multiply):
measured on HW, plain fp32 matmuls stream at 4 cycles/row and never release
the PE HAM clock throttle (1.2 GHz), capping PE at ~900 us/core -- 2.8x the
~320 us DMA roofline. float32r streams at 1 cycle/row (~150 us/core), putting
the kernel back on the memory roofline. Measured end-to-end: ~314 us/core
(slope-method HW timing; pure load+store DMA floor for the same tiling is
~283 us), max rel err vs the fp32 reference 1.8e-4.
"""

import numpy as np

import concourse.mybir as mybir
from concourse import bacc
from concourse.tile import TileContext
from concourse.bass_utils import run_bass_kernel_spmd

N_CORES = 8
B, C, H, W = 4, 128, 512, 512
NGROUP, CIN, COUT = 16, 8, 8
H_SH = H // N_CORES          # 64 rows per core
PIX = H_SH * W               # 32768 pixels per batch per core
TILE = 8192                  # f32 cols per DMA tile (32KB/partition, 4MB/DMA)
MM_N = 512                   # matmul free dim (one PSUM bank, fp32 max)
HALF = C // 2

_CACHE = {}


def _build_nc(repeat=1):
    key = ("nc", repeat)
    if key in _CACHE:
        return _CACHE[key]
    nc = bacc.Bacc()
    f32 = mybir.dt.float32
    f32r = mybir.dt.float32r
    x_d = nc.declare_dram_parameter("x", [B, C, PIX], f32r, isOutput=False)
    w_d = nc.declare_dram_parameter("wT", [C, C], f32r, isOutput=False)
    b_d = nc.declare_dram_parameter("bias", [C, 1], f32, isOutput=False)
    o_d = nc.declare_dram_parameter("out", [B, C, PIX], f32, isOutput=True)

    with TileContext(nc) as tc:
        with (
            tc.tile_pool(name="const", bufs=1) as cpool,
            tc.tile_pool(name="xin", bufs=2) as xpool,
            tc.tile_pool(name="oout", bufs=2) as opool,
            tc.tile_pool(name="ps", bufs=8, space="PSUM") as ppool,
        ):
            wt = cpool.tile([C, C], f32r)
            nc.sync.dma_start(out=wt, in_=w_d[:, :])
            bt = cpool.tile([C, 1], f32)
            nc.sync.dma_start(out=bt, in_=b_d[:, :])
            for _rep in range(repeat):
                for b in range(B):
                    for t in range(PIX // TILE):
                        xt = xpool.tile([C, TILE], f32r)
                        nc.sync.dma_start(
                            out=xt, in_=x_d[b, :, t * TILE:(t + 1) * TILE]
                        )
                        ot = opool.tile([C, TILE], f32)
                        for s in range(TILE // MM_N):
                            sl = slice(s * MM_N, (s + 1) * MM_N)
                            ps = ppool.tile([C, MM_N], f32)
                            nc.tensor.matmul(
                                ps, wt, xt[:, sl], start=True, stop=True
                            )
                            nc.any.tensor_scalar_add(ot[:, sl], ps, bt)
                        nc.sync.dma_start(
                            out=o_d[b, :, t * TILE:(t + 1) * TILE], in_=ot
                        )
    nc.finalize()
    _CACHE[key] = nc
    return nc


def _masked_wT(weight):
    go = np.arange(NGROUP * COUT) // COUT
    gi = np.arange(NGROUP * CIN) // CIN
    mask = (gi[None, :] <= go[:, None]).astype(np.float32)
    wt = weight.reshape(C, C) * mask          # [Cout, Cin]
    return np.ascontiguousarray(wt.T)         # [Cin, Cout] = lhsT


def kernel(x, weight, bias):
    x = np.asarray(x, dtype=np.float32)
    weight = np.asarray(weight, dtype=np.float32)
    bias = np.asarray(bias, dtype=np.float32)

    nc = _build_nc()
    wT = _masked_wT(weight)
    b2 = np.ascontiguousarray(bias.reshape(C, 1))

    in_maps = []
    for i in range(N_CORES):
        shard = np.ascontiguousarray(x[:, :, i * H_SH:(i + 1) * H_SH, :])
        in_maps.append({"x": shard.reshape(B, C, PIX), "wT": wT, "bias": b2})

    res = run_bass_kernel_spmd(nc, in_maps, core_ids=list(range(N_CORES)))

    out = np.empty((B, C, H, W), dtype=np.float32)
    for i in range(N_CORES):
        out[:, :, i * H_SH:(i + 1) * H_SH, :] = res.results[i]["out"].reshape(
            B, C, H_SH, W
        )
    return out
